# revision 16
# baseline (speedup 1.0000x reference)
"""DetectionLoss on 8 Trainium2 NeuronCores, data-parallel over the batch.

Algorithm
---------
The reference matches N=64512 grid anchors against M=20 gt boxes per image,
mines hard negatives by objectness loss, and reduces to 4 scalars. Direct
dense N x M IoU + per-image sort is compute-heavy; instead we use the grid
structure of the anchors:

  inter(h,w) = h_overlap[h] * w_overlap[w]          (separable per (a, m))
  iou >= t  <=>  R := inter/(area_a + area_g + eps) >= t/(1+t)   (monotone)

so  T := sum_m R_m^16  is computable with ONE small TensorE matmul per
(image, scale) from host-built 1-D tables, and  T < (3/13)^16  proves
best_iou < 0.3 ("definitely negative").  Anchors failing that test
(~250/image) are re-checked exactly on the host.

Device (per core: 4 images):
  * DMA packed objectness logits [128, 504] + tables,
  * TensorE: T via 7 rank-structured matmuls into one PSUM tile,
  * ScalarE: softplus(obj) (the negative-target BCE loss),
  * VectorE: candidate mask (u8), masked mining pool, per-scale
    per-partition top-16 via max8 + match_replace,
  * DMA out: mask [128,504] u8 + tops [128,48] bf16 per image.

Host: exact IoU for the few candidates (pos/neg classification, cls + loc
losses on ~30 positives/image), merges per-scale top-k mining sums from the
device tops, and performs the final scalar normalization. A per-(image,
scale) sufficiency check falls back to an exact host computation for that
scale if the per-partition top-16 could have missed part of the top-k (never
triggers for realistic data, guarantees exactness).
"""
import numpy as np

EPS = 1e-6
A = 3
C = 3
B = 32
P16 = 16
TAU_NEG_R = 3.0 / 13.0                    # R at iou == 0.3
TAU_CAND = 0.25 * TAU_NEG_R ** P16        # 4x safety margin on T
TOPP = 16                                 # exported per-partition top-k
NF = 504
N_CORES = 8
BPC = B // N_CORES                        # images per core

# per-scale layout per image (natural, p = h, f = a*W + w):
#   scale1: [128, 384], scale2: [64, 192], scale3: [32, 96]
SCALES = [dict(H=128, W=128), dict(H=64, W=64), dict(H=32, W=32)]
NSUM = [0, 49152, 61440, 64512]           # anchor-id bounds per scale
TOPS_N = [8, 8, 16]                       # exported per-partition top-k rounds*8
TOPS_OFF = [0, 8, 16]                     # column offsets in the packed tops tile
FCOL = [(0, 384), (384, 576), (576, 672)] # column ranges in the packed [128,672]
# table column layout within the per-image [60, 896] table tensor:
#   lhs_s = u16 [60, H], rhs_s = block-diag v16 [60, 3*W]
TCOLS = dict(l1=(0, 128), r1=(128, 512), l2=(512, 576), r2=(576, 768),
             l3=(768, 800), r3=(800, 896))


class _Fallback(Exception):
    pass


def _check_grid(anchors_list):
    """Verify separable grid anchors; return per-scale (x1,x2 [W,A], y1,y2 [H,A], aa [A])."""
    out = []
    for s, anc in enumerate(anchors_list):
        H, W = SCALES[s]['H'], SCALES[s]['W']
        if anc.shape != (H * W * A, 4):
            raise _Fallback
        a4 = anc.reshape(H, W, A, 4)
        x1 = a4[0, :, :, 0]; x2 = a4[0, :, :, 2]
        y1 = a4[:, 0, :, 1]; y2 = a4[:, 0, :, 3]
        rec = np.stack([np.broadcast_to(x1[None, :, :], (H, W, A)),
                        np.broadcast_to(y1[:, None, :], (H, W, A)),
                        np.broadcast_to(x2[None, :, :], (H, W, A)),
                        np.broadcast_to(y2[:, None, :], (H, W, A))], -1)
        if not np.array_equal(rec, a4):
            raise _Fallback
        aa = (x2[0] - x1[0]) * (y2[0] - y1[0])
        if (aa <= 0).any():
            raise _Fallback
        # v = w_overlap/sqrt(c) <= aspect bound; keep p=16 powers in f32 range
        if (np.abs(x2 - x1).max() > 1e4) or (np.abs(y2 - y1).max() > 1e4):
            raise _Fallback
        out.append((x1, x2, y1, y2, aa))
    return out


def _build_tables(grids, gt_boxes):
    """[B, 60, 896] f32 matmul tables (k = a*20 + m)."""
    M = gt_boxes.shape[1]
    if M != 20:
        raise _Fallback
    gx1, gy1, gx2, gy2 = [gt_boxes[..., i] for i in range(4)]   # [B, M]
    ag = (gx2 - gx1) * (gy2 - gy1)
    import ml_dtypes
    tabs = np.zeros((B, 60, 896), ml_dtypes.bfloat16)
    for s, (x1, x2, y1, y2, aa) in enumerate(grids):
        H, W = SCALES[s]['H'], SCALES[s]['W']
        c = aa[None, :, None] + ag[:, None, :] + EPS            # [B, A, M]
        if (c <= 0).any():
            raise _Fallback
        rc = (1.0 / np.sqrt(c))[..., None]
        wint = np.clip(np.minimum(x2.T[None, :, None, :], gx2[:, None, :, None])
                       - np.maximum(x1.T[None, :, None, :], gx1[:, None, :, None]), 0, None)
        hint = np.clip(np.minimum(y2.T[None, :, None, :], gy2[:, None, :, None])
                       - np.maximum(y1.T[None, :, None, :], gy1[:, None, :, None]), 0, None)
        u = ((hint * rc) ** P16).astype(np.float32)             # [B, A, M, H]
        v = ((wint * rc) ** P16).astype(np.float32)             # [B, A, M, W]
        lc = TCOLS[f'l{s+1}']; rcols = TCOLS[f'r{s+1}']
        tabs[:, :, lc[0]:lc[1]] = u.reshape(B, 60, H)
        for a in range(A):
            c0 = rcols[0] + a * W
            tabs[:, a * 20:(a + 1) * 20, c0:c0 + W] = v[:, a]
    return tabs


def _pack_obj(preds):
    """[B, 128, 672] bf16 objectness logits; scale s at [0:H, FCOL[s]] (f=a*W+w)."""
    import ml_dtypes
    out = np.zeros((B, 128, 672), ml_dtypes.bfloat16)
    for s, sc in enumerate(SCALES):
        H, W = sc['H'], sc['W']
        pl = preds[s].reshape(B, A, 8, H, W)[:, :, 4]           # [B, A, H, W]
        out[:, :H, FCOL[s][0]:FCOL[s][1]] = pl.transpose(0, 2, 1, 3).reshape(B, H, A * W)
    return out


def _unpack_idx():
    """Per scale: [H, 3*W] global anchor ids for the natural layout."""
    out = []
    for s, sc in enumerate(SCALES):
        H, W = sc['H'], sc['W']
        h = np.arange(H)[:, None, None]
        a = np.arange(A)[None, :, None]
        w = np.arange(W)[None, None, :]
        n = NSUM[s] + (h * W + w) * A + a
        out.append(n.reshape(H, A * W))
    return out


_NC_CACHE = {}
LAST_RESULTS = None


def _build_nc():
    import concourse.bass as bass
    import concourse.tile as tile
    import concourse.mybir as mybir
    from concourse import bacc

    bf16 = mybir.dt.bfloat16
    u8 = mybir.dt.uint8
    Alu = mybir.AluOpType

    nc = bacc.Bacc(None, target_bir_lowering=False)
    SH = [(128, 384), (64, 192), (32, 96)]
    obj_d = nc.dram_tensor("obj", [BPC, 128, 672], bf16, kind="ExternalInput")
    tab_d = nc.dram_tensor("tabs", [BPC, 60, 896], bf16, kind="ExternalInput")
    mask_d = [nc.dram_tensor(f"mask{s+1}", [BPC, H, F], u8, kind="ExternalOutput")
              for s, (H, F) in enumerate(SH)]
    tops_d = [nc.dram_tensor(f"tops{s+1}", [BPC, H, TOPS_N[s]], bf16,
                             kind="ExternalOutput")
              for s, (H, F) in enumerate(SH)]

    with tile.TileContext(nc) as tc:
        with tc.tile_pool(name="const", bufs=1) as cpool, \
             tc.tile_pool(name="sb", bufs=3) as pool, \
             tc.tile_pool(name="ps", bufs=2, space=bass.MemorySpace.PSUM) as psum:
            negc = cpool.tile([128, 1], bf16)
            nc.vector.memset(negc[:], -1e4)
            for i in range(BPC):
                obj_t = pool.tile([128, 672], bf16, tag="obj")
                nc.sync.dma_start(obj_t[:], obj_d[i])
                tab_t = pool.tile([60, 896], bf16, tag="tab")
                nc.sync.dma_start(tab_t[:], tab_d[i])
                for s, (H, F) in enumerate(SH):
                    f0, f1 = FCOL[s]
                    T = psum.tile([H, F], mybir.dt.float32, tag=f"T{s}")
                    lc = TCOLS[f'l{s+1}']; rc = TCOLS[f'r{s+1}']
                    nc.tensor.matmul(T[:], tab_t[:, lc[0]:lc[1]],
                                     tab_t[:, rc[0]:rc[1]], start=True, stop=True)
                    mask_t = pool.tile([H, F], u8, tag=f"mask{s}")
                    nc.vector.tensor_scalar(mask_t[:], T[:], TAU_CAND, None,
                                            op0=Alu.is_ge)
                    nc.sync.dma_start(mask_d[s][i], mask_t[:])
                    # mining pool in place: candidate logits -> -1e4
                    nc.vector.copy_predicated(obj_t[:H, f0:f1], mask_t[:],
                                              negc[:H].to_broadcast([H, F]))
                    tops_t = pool.tile([H, TOPS_N[s]], bf16, tag=f"tops{s}")
                    for r in range(TOPS_N[s] // 8):
                        if r > 0:
                            nc.vector.match_replace(
                                out=obj_t[:H, f0:f1],
                                in_to_replace=tops_t[:, 8*r-8:8*r],
                                in_values=obj_t[:H, f0:f1], imm_value=-1e4)
                        nc.vector.max(tops_t[:, 8*r:8*r+8], obj_t[:H, f0:f1])
                    nc.sync.dma_start(tops_d[s][i], tops_t[:])
    nc.finalize()
    return nc


def _run_device(objpack, tabs, trace=False):
    from concourse.bass_utils import run_bass_kernel_spmd
    global LAST_RESULTS
    if 'nc' not in _NC_CACHE:
        _NC_CACHE['nc'] = _build_nc()
    in_maps = []
    for i in range(N_CORES):
        sl = slice(i * BPC, (i + 1) * BPC)
        in_maps.append({"obj": np.ascontiguousarray(objpack[sl]),
                        "tabs": np.ascontiguousarray(tabs[sl])})
    res = run_bass_kernel_spmd(_NC_CACHE['nc'], in_maps, list(range(N_CORES)),
                               trace=trace)
    LAST_RESULTS = res
    masks = [np.concatenate([np.asarray(r[f"mask{s+1}"]) for r in res.results], 0)
             for s in range(3)]
    tops = [np.concatenate([np.asarray(r[f"tops{s+1}"]).astype(np.float32)
                            for r in res.results], 0)
            for s in range(3)]
    return masks, tops


def _softplus(x):
    return np.log1p(np.exp(-np.abs(x))) + np.maximum(x, 0)


def _host_finish(inputs, masks, tops_all):
    anchors = np.concatenate([inputs[f'anchors{i}'] for i in (1, 2, 3)], 0)
    aa = (anchors[:, 2] - anchors[:, 0]) * (anchors[:, 3] - anchors[:, 1])
    idx_maps = _unpack_idx()
    preds = [inputs['pred1'], inputs['pred2'], inputs['pred3']]
    pflat = [p.reshape(B, 24, -1) for p in preds]
    obj_sum = 0.0; obj_den = 0; cls_sum = 0.0; loc_sum = 0.0; n_pos_t = 0
    for b in range(B):
        gt = inputs['gt_boxes'][b]; lab = inputs['gt_labels'][b]
        ag = (gt[:, 2] - gt[:, 0]) * (gt[:, 3] - gt[:, 1])
        cand_n = np.concatenate([idx_maps[s][masks[s][b] > 0] for s in range(3)])
        ca = anchors[cand_n]
        lt = np.maximum(ca[:, None, :2], gt[None, :, :2])
        rb = np.minimum(ca[:, None, 2:], gt[None, :, 2:])
        wh = np.clip(rb - lt, 0, None)
        inter = wh[..., 0] * wh[..., 1]
        iou = inter / (aa[cand_n][:, None] + ag[None, :] - inter + EPS)
        bi = iou.max(1) if cand_n.size else np.empty(0)
        bg = iou.argmax(1) if cand_n.size else np.empty(0, np.int64)
        pos_c = bi >= 0.5
        nonneg_c = bi >= 0.3
        pos_n = cand_n[pos_c]; pos_bg = bg[pos_c]
        n_pos = pos_n.size

        def gather(ns, chans):
            out = np.empty((len(chans), ns.size), np.float32)
            for s in range(3):
                m = (ns >= NSUM[s]) & (ns < NSUM[s + 1])
                if not m.any():
                    continue
                loc = ns[m] - NSUM[s]
                a = loc % A; hw = loc // A
                for ci, c in enumerate(chans):
                    out[ci, m] = pflat[s][b][a * 8 + c, hw]
            return out

        for s in range(3):
            in_s = (cand_n >= NSUM[s]) & (cand_n < NSUM[s + 1])
            n_pos_s = int((pos_c & in_s).sum())
            k = 3 * max(n_pos_s, 1)
            n_neg_s = (NSUM[s + 1] - NSUM[s]) - int((nonneg_c & in_s).sum())
            k_eff = min(k, n_neg_s)
            cn = cand_n[(~nonneg_c) & in_s]
            extra = _softplus(gather(cn, [4])[0]) if cn.size else np.empty(0, np.float32)
            tseg = tops_all[s][b]                # raw logits, -1e4 = excluded
            merged = np.concatenate([_softplus(tseg[tseg > -1e3].ravel()), extra])
            sel = np.sort(merged)[::-1][:k_eff]
            tstar = sel[-1] if (k_eff > 0 and sel.size == k_eff) else np.inf
            if (sel.size < k_eff) or (not np.isfinite(sel.sum())) \
                    or np.any(_softplus(tseg[:, -1]) >= tstar):
                # exact fallback for this (image, scale)
                x_all = pflat[s][b][[4, 12, 20]].T.ravel()      # n_local = hw*A + a
                sp_all = _softplus(x_all)
                negm = np.ones(NSUM[s + 1] - NSUM[s], bool)
                negm[cand_n[nonneg_c & in_s] - NSUM[s]] = False
                sel = np.sort(sp_all[negm])[::-1][:k_eff]
            obj_sum += float(sel.sum())
            obj_den += n_pos_s + k_eff

        if n_pos:
            pv = gather(pos_n, [4, 5, 6, 7, 0, 1, 2, 3])
            x = pv[0]
            obj_sum += float((_softplus(x) - x).sum())
            logits = pv[1:4]
            mlog = logits.max(0)
            lse = mlog + np.log(np.exp(logits - mlog).sum(0))
            tgt = np.clip(lab[pos_bg], 0, C - 1)
            cls_sum += float((lse - logits[tgt, np.arange(n_pos)]).sum())
            mb = gt[pos_bg]; anc = anchors[pos_n]

            def cxcywh(bx):
                w = np.maximum(bx[:, 2] - bx[:, 0], EPS)
                h = np.maximum(bx[:, 3] - bx[:, 1], EPS)
                return bx[:, 0] + 0.5 * w, bx[:, 1] + 0.5 * h, w, h

            gcx, gcy, gw, gh = cxcywh(mb)
            acx, acy, aw, ah = cxcywh(anc)
            t = np.stack([(gcx - acx) / (aw + EPS), (gcy - acy) / (ah + EPS),
                          np.log((gw + EPS) / (aw + EPS)),
                          np.log((gh + EPS) / (ah + EPS))])
            d = pv[4:8] - t
            ad = np.abs(d)
            loc_sum += float(np.where(ad < 1, 0.5 * d * d, ad - 0.5).sum())
        n_pos_t += n_pos

    pos_norm = max(n_pos_t, 1); obj_norm = max(obj_den, 1)
    lo = obj_sum / obj_norm; lc = cls_sum / pos_norm; ll = loc_sum / pos_norm
    return np.array([lo, lc, ll, lo + lc + 2 * ll], np.float32)


def _kernel_numpy(pred1, pred2, pred3, anchors1, anchors2, anchors3,
                  gt_boxes, gt_labels):
    """Exact reference-equivalent numpy fallback (arbitrary inputs)."""
    tot = [0.0, 0, 0.0, 0.0, 0]
    for pred, anc in ((pred1, anchors1), (pred2, anchors2), (pred3, anchors3)):
        Bb, ch, H, W = pred.shape
        p = pred.reshape(Bb, A, 5 + C, H, W).transpose(0, 3, 4, 1, 2).reshape(Bb, -1, 5 + C)
        N = p.shape[1]
        aa = (anc[:, 2] - anc[:, 0]) * (anc[:, 3] - anc[:, 1])
        for b in range(Bb):
            boxes = gt_boxes[b]; labels = gt_labels[b]
            ag = (boxes[:, 2] - boxes[:, 0]) * (boxes[:, 3] - boxes[:, 1])
            lt = np.maximum(anc[:, None, :2], boxes[None, :, :2])
            rb = np.minimum(anc[:, None, 2:], boxes[None, :, 2:])
            wh = np.clip(rb - lt, 0, None)
            inter = wh[..., 0] * wh[..., 1]
            ious = inter / (aa[:, None] + ag[None, :] - inter + EPS)
            bi = ious.max(1); bg = ious.argmax(1)
            pos = bi >= 0.5; neg = bi < 0.3
            x = p[b, :, 4]
            ol = np.maximum(x, 0) - x * pos + np.log1p(np.exp(-np.abs(x)))
            k = 3 * max(int(pos.sum()), 1)
            nl = np.where(neg, ol, -np.inf)
            order = np.argsort(-nl, kind='stable')
            rank = np.empty(N, np.int64); rank[order] = np.arange(N)
            seln = (rank < k) & neg
            m = pos | seln
            tot[0] += float(ol[m].sum()); tot[1] += int(m.sum())
            if pos.any():
                logits = p[b, pos, 5:]
                mlog = logits.max(1, keepdims=True)
                lse = (mlog[:, 0] + np.log(np.exp(logits - mlog).sum(1)))
                tgt = np.clip(labels[bg[pos]], 0, C - 1)
                tot[2] += float((lse - logits[np.arange(tgt.size), tgt]).sum())
                mb = boxes[bg[pos]]; ap_ = anc[pos]

                def cxcywh(bx):
                    w = np.maximum(bx[:, 2] - bx[:, 0], EPS)
                    h = np.maximum(bx[:, 3] - bx[:, 1], EPS)
                    return bx[:, 0] + 0.5 * w, bx[:, 1] + 0.5 * h, w, h

                gcx, gcy, gw, gh = cxcywh(mb); acx, acy, aw, ah = cxcywh(ap_)
                t = np.stack([(gcx - acx) / (aw + EPS), (gcy - acy) / (ah + EPS),
                              np.log((gw + EPS) / (aw + EPS)),
                              np.log((gh + EPS) / (ah + EPS))], 1)
                d = p[b, pos, :4] - t
                ad = np.abs(d)
                tot[3] += float(np.where(ad < 1, 0.5 * d * d, ad - 0.5).sum())
                tot[4] += int(pos.sum())
    lo = tot[0] / max(tot[1], 1); lc = tot[2] / max(tot[4], 1)
    ll = tot[3] / max(tot[4], 1)
    return np.array([lo, lc, ll, lo + lc + 2 * ll], np.float32)


def kernel(pred1, pred2, pred3, anchors1, anchors2, anchors3,
           gt_boxes, gt_labels, _trace=False):
    args = dict(pred1=np.asarray(pred1, np.float32),
                pred2=np.asarray(pred2, np.float32),
                pred3=np.asarray(pred3, np.float32),
                anchors1=np.asarray(anchors1, np.float32),
                anchors2=np.asarray(anchors2, np.float32),
                anchors3=np.asarray(anchors3, np.float32),
                gt_boxes=np.asarray(gt_boxes, np.float32),
                gt_labels=np.asarray(gt_labels))
    try:
        if args['pred1'].shape != (B, 24, 128, 128) or args['gt_boxes'].shape != (B, 20, 4):
            raise _Fallback
        grids = _check_grid([args[f'anchors{i}'] for i in (1, 2, 3)])
        tabs = _build_tables(grids, args['gt_boxes'])
    except _Fallback:
        return _kernel_numpy(**{k: v for k, v in args.items()})
    objpack = _pack_obj([args[f'pred{i}'] for i in (1, 2, 3)])
    masks, tops = _run_device(objpack, tabs, trace=_trace)
    return _host_finish(args, masks, tops)


# revision 19
# speedup vs baseline: 1.1556x; 1.1556x over previous
"""DetectionLoss on 8 Trainium2 NeuronCores, data-parallel over the batch.

Algorithm
---------
The reference matches N=64512 grid anchors against M=20 gt boxes per image,
mines hard negatives by objectness loss, and reduces to 4 scalars. Direct
dense N x M IoU + per-image sort is compute-heavy; instead we use the grid
structure of the anchors:

  inter(h,w) = h_overlap[h] * w_overlap[w]          (separable per (a, m))
  iou >= t  <=>  R := inter/(area_a + area_g + eps) >= t/(1+t)   (monotone)

so  T := sum_m R_m^16  is computable with ONE small TensorE matmul per
(image, scale) from host-built 1-D tables, and  T < (3/13)^16  proves
best_iou < 0.3 ("definitely negative").  Anchors failing that test
(~250/image) are re-checked exactly on the host.

Device (per core: 4 images):
  * DMA packed objectness logits [128, 504] + tables,
  * TensorE: T via 7 rank-structured matmuls into one PSUM tile,
  * ScalarE: softplus(obj) (the negative-target BCE loss),
  * VectorE: candidate mask (u8), masked mining pool, per-scale
    per-partition top-16 via max8 + match_replace,
  * DMA out: mask [128,504] u8 + tops [128,48] bf16 per image.

Host: exact IoU for the few candidates (pos/neg classification, cls + loc
losses on ~30 positives/image), merges per-scale top-k mining sums from the
device tops, and performs the final scalar normalization. A per-(image,
scale) sufficiency check falls back to an exact host computation for that
scale if the per-partition top-16 could have missed part of the top-k (never
triggers for realistic data, guarantees exactness).
"""
import numpy as np

EPS = 1e-6
A = 3
C = 3
B = 32
P16 = 16
TAU_NEG_R = 3.0 / 13.0                    # R at iou == 0.3
TAU_CAND = 0.25 * TAU_NEG_R ** P16        # 4x safety margin on T
TOPP = 16                                 # exported per-partition top-k
NF = 504
N_CORES = 8
BPC = B // N_CORES                        # images per core

# per-scale layout per image (natural, p = h, f = a*W + w):
#   scale1: [128, 384], scale2: [64, 192], scale3: [32, 96]
SCALES = [dict(H=128, W=128), dict(H=64, W=64), dict(H=32, W=32)]
NSUM = [0, 49152, 61440, 64512]           # anchor-id bounds per scale
TOPS_N = [8, 8, 16]                       # exported per-partition top-k rounds*8
TOPS_OFF = [0, 8, 16]                     # column offsets in the packed tops tile
FCOL = [(0, 384), (384, 576), (576, 672)] # column ranges in the packed [128,672]
# table column layout within the per-image [60, 896] table tensor:
#   lhs_s = u16 [60, H], rhs_s = block-diag v16 [60, 3*W]
TCOLS = dict(l1=(0, 128), r1=(128, 512), l2=(512, 576), r2=(576, 768),
             l3=(768, 800), r3=(800, 896))


class _Fallback(Exception):
    pass


def _check_grid(anchors_list):
    """Verify separable grid anchors; return per-scale (x1,x2 [W,A], y1,y2 [H,A], aa [A])."""
    out = []
    for s, anc in enumerate(anchors_list):
        H, W = SCALES[s]['H'], SCALES[s]['W']
        if anc.shape != (H * W * A, 4):
            raise _Fallback
        a4 = anc.reshape(H, W, A, 4)
        x1 = a4[0, :, :, 0]; x2 = a4[0, :, :, 2]
        y1 = a4[:, 0, :, 1]; y2 = a4[:, 0, :, 3]
        rec = np.stack([np.broadcast_to(x1[None, :, :], (H, W, A)),
                        np.broadcast_to(y1[:, None, :], (H, W, A)),
                        np.broadcast_to(x2[None, :, :], (H, W, A)),
                        np.broadcast_to(y2[:, None, :], (H, W, A))], -1)
        if not np.array_equal(rec, a4):
            raise _Fallback
        aa = (x2[0] - x1[0]) * (y2[0] - y1[0])
        if (aa <= 0).any():
            raise _Fallback
        # v = w_overlap/sqrt(c) <= aspect bound; keep p=16 powers in f32 range
        if (np.abs(x2 - x1).max() > 1e4) or (np.abs(y2 - y1).max() > 1e4):
            raise _Fallback
        out.append((x1, x2, y1, y2, aa))
    return out


def _build_tables(grids, gt_boxes):
    """[B, 60, 896] f32 matmul tables (k = a*20 + m)."""
    M = gt_boxes.shape[1]
    if M != 20:
        raise _Fallback
    gx1, gy1, gx2, gy2 = [gt_boxes[..., i] for i in range(4)]   # [B, M]
    ag = (gx2 - gx1) * (gy2 - gy1)
    import ml_dtypes
    tabs = np.zeros((B, 60, 896), ml_dtypes.bfloat16)
    for s, (x1, x2, y1, y2, aa) in enumerate(grids):
        H, W = SCALES[s]['H'], SCALES[s]['W']
        c = aa[None, :, None] + ag[:, None, :] + EPS            # [B, A, M]
        if (c <= 0).any():
            raise _Fallback
        rc = (1.0 / np.sqrt(c))[..., None]
        wint = np.clip(np.minimum(x2.T[None, :, None, :], gx2[:, None, :, None])
                       - np.maximum(x1.T[None, :, None, :], gx1[:, None, :, None]), 0, None)
        hint = np.clip(np.minimum(y2.T[None, :, None, :], gy2[:, None, :, None])
                       - np.maximum(y1.T[None, :, None, :], gy1[:, None, :, None]), 0, None)
        u = ((hint * rc) ** P16).astype(np.float32)             # [B, A, M, H]
        v = ((wint * rc) ** P16).astype(np.float32)             # [B, A, M, W]
        lc = TCOLS[f'l{s+1}']; rcols = TCOLS[f'r{s+1}']
        tabs[:, :, lc[0]:lc[1]] = u.reshape(B, 60, H)
        for a in range(A):
            c0 = rcols[0] + a * W
            tabs[:, a * 20:(a + 1) * 20, c0:c0 + W] = v[:, a]
    return tabs


def _pack_obj(preds):
    """[B, 128, 672] bf16 objectness logits; scale s at [0:H, FCOL[s]] (f=a*W+w)."""
    import ml_dtypes
    out = np.zeros((B, 128, 672), ml_dtypes.bfloat16)
    for s, sc in enumerate(SCALES):
        H, W = sc['H'], sc['W']
        pl = preds[s].reshape(B, A, 8, H, W)[:, :, 4]           # [B, A, H, W]
        out[:, :H, FCOL[s][0]:FCOL[s][1]] = pl.transpose(0, 2, 1, 3).reshape(B, H, A * W)
    return out


def _unpack_idx():
    """Per scale: [H, 3*W] global anchor ids for the natural layout."""
    out = []
    for s, sc in enumerate(SCALES):
        H, W = sc['H'], sc['W']
        h = np.arange(H)[:, None, None]
        a = np.arange(A)[None, :, None]
        w = np.arange(W)[None, None, :]
        n = NSUM[s] + (h * W + w) * A + a
        out.append(n.reshape(H, A * W))
    return out


_NC_CACHE = {}
LAST_RESULTS = None


def _build_nc():
    import concourse.bass as bass
    import concourse.tile as tile
    import concourse.mybir as mybir
    from concourse import bacc

    f32 = mybir.dt.float32
    bf16 = mybir.dt.bfloat16
    Alu = mybir.AluOpType

    nc = bacc.Bacc(None, target_bir_lowering=False)
    SH = [(128, 384), (64, 192), (32, 96)]
    PBANK = [512, 256, 128]        # per-image PSUM column stride (f32, bank-aligned)
    obj_d = nc.dram_tensor("obj", [128, BPC, 672], bf16, kind="ExternalInput")
    tab_d = nc.dram_tensor("tabs", [60, BPC, 896], bf16, kind="ExternalInput")
    excl_d = nc.dram_tensor("excl", [128, BPC, 672], bf16, kind="ExternalOutput")
    tops_d = nc.dram_tensor("tops", [128, BPC, 32], bf16, kind="ExternalOutput")

    with tile.TileContext(nc) as tc:
        with tc.tile_pool(name="sb", bufs=1) as pool, \
             tc.tile_pool(name="ps", bufs=1, space=bass.MemorySpace.PSUM) as psum:
            obj_t = pool.tile([128, BPC, 672], bf16)
            nc.sync.dma_start(obj_t[:], obj_d[:])
            tab_t = pool.tile([60, BPC, 896], bf16)
            nc.sync.dma_start(tab_t[:], tab_d[:])
            excl_t = pool.tile([128, BPC, 672], bf16)
            tops_t = pool.tile([128, BPC, 32], bf16)
            # rows no scale writes, so packed DMA-out reads initialized tiles
            nc.vector.memset(excl_t[64:, :, 384:576], 0)
            nc.vector.memset(excl_t[32:64, :, 576:672], 0)
            nc.vector.memset(excl_t[64:, :, 576:672], 0)
            nc.vector.memset(tops_t[64:, :, 8:16], 0)
            nc.vector.memset(tops_t[32:64, :, 16:32], 0)
            nc.vector.memset(tops_t[64:, :, 16:32], 0)

            Ts = [psum.tile([SH[s][0], BPC, PBANK[s]], f32, name=f"T{s}")
                  for s in range(3)]
            for s, (H, F) in enumerate(SH):
                lc = TCOLS[f'l{s+1}']; rc = TCOLS[f'r{s+1}']
                for i in range(BPC):
                    nc.tensor.matmul(Ts[s][:, i, 0:F],
                                     tab_t[:, i, lc[0]:lc[1]],
                                     tab_t[:, i, rc[0]:rc[1]],
                                     start=True, stop=True)
            for s, (H, F) in enumerate(SH):
                f0, f1 = FCOL[s]
                # excl = -1e4 where T >= tau (exported as the candidate mask)
                nc.vector.tensor_scalar(excl_t[:H, :, f0:f1], Ts[s][:, :, 0:F],
                                        TAU_CAND, -1e4, op0=Alu.is_ge, op1=Alu.mult)
                # mining pool in place: candidate logits -> x - 1e4
                nc.vector.tensor_add(obj_t[:H, :, f0:f1], obj_t[:H, :, f0:f1],
                                     excl_t[:H, :, f0:f1])
                t0 = TOPS_OFF[s]
                for i in range(BPC):
                    for r in range(TOPS_N[s] // 8):
                        if r > 0:
                            nc.vector.match_replace(
                                out=obj_t[:H, i, f0:f1],
                                in_to_replace=tops_t[:H, i, t0 + 8*r-8:t0 + 8*r],
                                in_values=obj_t[:H, i, f0:f1], imm_value=-1e4)
                        nc.vector.max(tops_t[:H, i, t0 + 8*r:t0 + 8*r+8],
                                      obj_t[:H, i, f0:f1])
            nc.sync.dma_start(excl_d[:], excl_t[:])
            nc.sync.dma_start(tops_d[:], tops_t[:])
    nc.finalize()
    return nc


def _run_device(objpack, tabs, trace=False):
    from concourse.bass_utils import run_bass_kernel_spmd
    global LAST_RESULTS
    if 'nc' not in _NC_CACHE:
        _NC_CACHE['nc'] = _build_nc()
    in_maps = []
    for i in range(N_CORES):
        sl = slice(i * BPC, (i + 1) * BPC)
        in_maps.append({"obj": np.ascontiguousarray(objpack[sl].transpose(1, 0, 2)),
                        "tabs": np.ascontiguousarray(tabs[sl].transpose(1, 0, 2))})
    res = run_bass_kernel_spmd(_NC_CACHE['nc'], in_maps, list(range(N_CORES)),
                               trace=trace)
    LAST_RESULTS = res
    # back to [B, 128, 672] / [B, 128, 32] image-major views
    excl = np.concatenate([np.asarray(r["excl"]).astype(np.float32).transpose(1, 0, 2)
                           for r in res.results], 0)
    tops = np.concatenate([np.asarray(r["tops"]).astype(np.float32).transpose(1, 0, 2)
                           for r in res.results], 0)
    masks = [excl[:, :SCALES[s]['H'], FCOL[s][0]:FCOL[s][1]] < -1.0 for s in range(3)]
    tops_l = [tops[:, :SCALES[s]['H'], TOPS_OFF[s]:TOPS_OFF[s] + TOPS_N[s]]
              for s in range(3)]
    return masks, tops_l


def _softplus(x):
    return np.log1p(np.exp(-np.abs(x))) + np.maximum(x, 0)


def _host_finish(inputs, masks, tops_all):
    anchors = np.concatenate([inputs[f'anchors{i}'] for i in (1, 2, 3)], 0)
    aa = (anchors[:, 2] - anchors[:, 0]) * (anchors[:, 3] - anchors[:, 1])
    idx_maps = _unpack_idx()
    preds = [inputs['pred1'], inputs['pred2'], inputs['pred3']]
    pflat = [p.reshape(B, 24, -1) for p in preds]
    obj_sum = 0.0; obj_den = 0; cls_sum = 0.0; loc_sum = 0.0; n_pos_t = 0
    for b in range(B):
        gt = inputs['gt_boxes'][b]; lab = inputs['gt_labels'][b]
        ag = (gt[:, 2] - gt[:, 0]) * (gt[:, 3] - gt[:, 1])
        cand_n = np.concatenate([idx_maps[s][masks[s][b] > 0] for s in range(3)])
        ca = anchors[cand_n]
        lt = np.maximum(ca[:, None, :2], gt[None, :, :2])
        rb = np.minimum(ca[:, None, 2:], gt[None, :, 2:])
        wh = np.clip(rb - lt, 0, None)
        inter = wh[..., 0] * wh[..., 1]
        iou = inter / (aa[cand_n][:, None] + ag[None, :] - inter + EPS)
        bi = iou.max(1) if cand_n.size else np.empty(0)
        bg = iou.argmax(1) if cand_n.size else np.empty(0, np.int64)
        pos_c = bi >= 0.5
        nonneg_c = bi >= 0.3
        pos_n = cand_n[pos_c]; pos_bg = bg[pos_c]
        n_pos = pos_n.size

        def gather(ns, chans):
            out = np.empty((len(chans), ns.size), np.float32)
            for s in range(3):
                m = (ns >= NSUM[s]) & (ns < NSUM[s + 1])
                if not m.any():
                    continue
                loc = ns[m] - NSUM[s]
                a = loc % A; hw = loc // A
                for ci, c in enumerate(chans):
                    out[ci, m] = pflat[s][b][a * 8 + c, hw]
            return out

        for s in range(3):
            in_s = (cand_n >= NSUM[s]) & (cand_n < NSUM[s + 1])
            n_pos_s = int((pos_c & in_s).sum())
            k = 3 * max(n_pos_s, 1)
            n_neg_s = (NSUM[s + 1] - NSUM[s]) - int((nonneg_c & in_s).sum())
            k_eff = min(k, n_neg_s)
            cn = cand_n[(~nonneg_c) & in_s]
            extra = _softplus(gather(cn, [4])[0]) if cn.size else np.empty(0, np.float32)
            tseg = tops_all[s][b]                # raw logits, -1e4 = excluded
            merged = np.concatenate([_softplus(tseg[tseg > -1e3].ravel()), extra])
            sel = np.sort(merged)[::-1][:k_eff]
            tstar = sel[-1] if (k_eff > 0 and sel.size == k_eff) else np.inf
            if (sel.size < k_eff) or (not np.isfinite(sel.sum())) \
                    or np.any(_softplus(tseg[:, -1]) >= tstar):
                # exact fallback for this (image, scale)
                x_all = pflat[s][b][[4, 12, 20]].T.ravel()      # n_local = hw*A + a
                sp_all = _softplus(x_all)
                negm = np.ones(NSUM[s + 1] - NSUM[s], bool)
                negm[cand_n[nonneg_c & in_s] - NSUM[s]] = False
                sel = np.sort(sp_all[negm])[::-1][:k_eff]
            obj_sum += float(sel.sum())
            obj_den += n_pos_s + k_eff

        if n_pos:
            pv = gather(pos_n, [4, 5, 6, 7, 0, 1, 2, 3])
            x = pv[0]
            obj_sum += float((_softplus(x) - x).sum())
            logits = pv[1:4]
            mlog = logits.max(0)
            lse = mlog + np.log(np.exp(logits - mlog).sum(0))
            tgt = np.clip(lab[pos_bg], 0, C - 1)
            cls_sum += float((lse - logits[tgt, np.arange(n_pos)]).sum())
            mb = gt[pos_bg]; anc = anchors[pos_n]

            def cxcywh(bx):
                w = np.maximum(bx[:, 2] - bx[:, 0], EPS)
                h = np.maximum(bx[:, 3] - bx[:, 1], EPS)
                return bx[:, 0] + 0.5 * w, bx[:, 1] + 0.5 * h, w, h

            gcx, gcy, gw, gh = cxcywh(mb)
            acx, acy, aw, ah = cxcywh(anc)
            t = np.stack([(gcx - acx) / (aw + EPS), (gcy - acy) / (ah + EPS),
                          np.log((gw + EPS) / (aw + EPS)),
                          np.log((gh + EPS) / (ah + EPS))])
            d = pv[4:8] - t
            ad = np.abs(d)
            loc_sum += float(np.where(ad < 1, 0.5 * d * d, ad - 0.5).sum())
        n_pos_t += n_pos

    pos_norm = max(n_pos_t, 1); obj_norm = max(obj_den, 1)
    lo = obj_sum / obj_norm; lc = cls_sum / pos_norm; ll = loc_sum / pos_norm
    return np.array([lo, lc, ll, lo + lc + 2 * ll], np.float32)


def _kernel_numpy(pred1, pred2, pred3, anchors1, anchors2, anchors3,
                  gt_boxes, gt_labels):
    """Exact reference-equivalent numpy fallback (arbitrary inputs)."""
    tot = [0.0, 0, 0.0, 0.0, 0]
    for pred, anc in ((pred1, anchors1), (pred2, anchors2), (pred3, anchors3)):
        Bb, ch, H, W = pred.shape
        p = pred.reshape(Bb, A, 5 + C, H, W).transpose(0, 3, 4, 1, 2).reshape(Bb, -1, 5 + C)
        N = p.shape[1]
        aa = (anc[:, 2] - anc[:, 0]) * (anc[:, 3] - anc[:, 1])
        for b in range(Bb):
            boxes = gt_boxes[b]; labels = gt_labels[b]
            ag = (boxes[:, 2] - boxes[:, 0]) * (boxes[:, 3] - boxes[:, 1])
            lt = np.maximum(anc[:, None, :2], boxes[None, :, :2])
            rb = np.minimum(anc[:, None, 2:], boxes[None, :, 2:])
            wh = np.clip(rb - lt, 0, None)
            inter = wh[..., 0] * wh[..., 1]
            ious = inter / (aa[:, None] + ag[None, :] - inter + EPS)
            bi = ious.max(1); bg = ious.argmax(1)
            pos = bi >= 0.5; neg = bi < 0.3
            x = p[b, :, 4]
            ol = np.maximum(x, 0) - x * pos + np.log1p(np.exp(-np.abs(x)))
            k = 3 * max(int(pos.sum()), 1)
            nl = np.where(neg, ol, -np.inf)
            order = np.argsort(-nl, kind='stable')
            rank = np.empty(N, np.int64); rank[order] = np.arange(N)
            seln = (rank < k) & neg
            m = pos | seln
            tot[0] += float(ol[m].sum()); tot[1] += int(m.sum())
            if pos.any():
                logits = p[b, pos, 5:]
                mlog = logits.max(1, keepdims=True)
                lse = (mlog[:, 0] + np.log(np.exp(logits - mlog).sum(1)))
                tgt = np.clip(labels[bg[pos]], 0, C - 1)
                tot[2] += float((lse - logits[np.arange(tgt.size), tgt]).sum())
                mb = boxes[bg[pos]]; ap_ = anc[pos]

                def cxcywh(bx):
                    w = np.maximum(bx[:, 2] - bx[:, 0], EPS)
                    h = np.maximum(bx[:, 3] - bx[:, 1], EPS)
                    return bx[:, 0] + 0.5 * w, bx[:, 1] + 0.5 * h, w, h

                gcx, gcy, gw, gh = cxcywh(mb); acx, acy, aw, ah = cxcywh(ap_)
                t = np.stack([(gcx - acx) / (aw + EPS), (gcy - acy) / (ah + EPS),
                              np.log((gw + EPS) / (aw + EPS)),
                              np.log((gh + EPS) / (ah + EPS))], 1)
                d = p[b, pos, :4] - t
                ad = np.abs(d)
                tot[3] += float(np.where(ad < 1, 0.5 * d * d, ad - 0.5).sum())
                tot[4] += int(pos.sum())
    lo = tot[0] / max(tot[1], 1); lc = tot[2] / max(tot[4], 1)
    ll = tot[3] / max(tot[4], 1)
    return np.array([lo, lc, ll, lo + lc + 2 * ll], np.float32)


def kernel(pred1, pred2, pred3, anchors1, anchors2, anchors3,
           gt_boxes, gt_labels, _trace=False):
    args = dict(pred1=np.asarray(pred1, np.float32),
                pred2=np.asarray(pred2, np.float32),
                pred3=np.asarray(pred3, np.float32),
                anchors1=np.asarray(anchors1, np.float32),
                anchors2=np.asarray(anchors2, np.float32),
                anchors3=np.asarray(anchors3, np.float32),
                gt_boxes=np.asarray(gt_boxes, np.float32),
                gt_labels=np.asarray(gt_labels))
    try:
        if args['pred1'].shape != (B, 24, 128, 128) or args['gt_boxes'].shape != (B, 20, 4):
            raise _Fallback
        grids = _check_grid([args[f'anchors{i}'] for i in (1, 2, 3)])
        tabs = _build_tables(grids, args['gt_boxes'])
    except _Fallback:
        return _kernel_numpy(**{k: v for k, v in args.items()})
    objpack = _pack_obj([args[f'pred{i}'] for i in (1, 2, 3)])
    masks, tops = _run_device(objpack, tabs, trace=_trace)
    return _host_finish(args, masks, tops)


# revision 20
# speedup vs baseline: 1.2201x; 1.0558x over previous
"""DetectionLoss on 8 Trainium2 NeuronCores, data-parallel over the batch.

Algorithm
---------
The reference matches N=64512 grid anchors against M=20 gt boxes per image,
mines hard negatives by objectness loss, and reduces to 4 scalars. Direct
dense N x M IoU + per-image sort is compute-heavy; instead we use the grid
structure of the anchors:

  inter(h,w) = h_overlap[h] * w_overlap[w]          (separable per (a, m))
  iou >= t  <=>  R := inter/(area_a + area_g + eps) >= t/(1+t)   (monotone)

so  T := sum_m R_m^16  is computable with ONE small TensorE matmul per
(image, scale) from host-built 1-D tables, and  T < (3/13)^16  proves
best_iou < 0.3 ("definitely negative").  Anchors failing that test
(~250/image) are re-checked exactly on the host.

Device (per core: 4 images):
  * DMA packed objectness logits [128, 504] + tables,
  * TensorE: T via 7 rank-structured matmuls into one PSUM tile,
  * ScalarE: softplus(obj) (the negative-target BCE loss),
  * VectorE: candidate mask (u8), masked mining pool, per-scale
    per-partition top-16 via max8 + match_replace,
  * DMA out: mask [128,504] u8 + tops [128,48] bf16 per image.

Host: exact IoU for the few candidates (pos/neg classification, cls + loc
losses on ~30 positives/image), merges per-scale top-k mining sums from the
device tops, and performs the final scalar normalization. A per-(image,
scale) sufficiency check falls back to an exact host computation for that
scale if the per-partition top-16 could have missed part of the top-k (never
triggers for realistic data, guarantees exactness).
"""
import numpy as np

EPS = 1e-6
A = 3
C = 3
B = 32
P16 = 16
TAU_NEG_R = 3.0 / 13.0                    # R at iou == 0.3
TAU_CAND = 0.25 * TAU_NEG_R ** P16        # 4x safety margin on T
TOPP = 16                                 # exported per-partition top-k
NF = 504
N_CORES = 8
BPC = B // N_CORES                        # images per core

# per-scale layout per image (natural, p = h, f = a*W + w):
#   scale1: [128, 384], scale2: [64, 192], scale3: [32, 96]
SCALES = [dict(H=128, W=128), dict(H=64, W=64), dict(H=32, W=32)]
NSUM = [0, 49152, 61440, 64512]           # anchor-id bounds per scale
TOPS_N = [8, 8, 8]                        # exported per-partition top-k rounds*8
TOPS_OFF = [0, 8, 16]                     # column offsets in the packed tops tile
FCOL = [(0, 384), (384, 576), (576, 672)] # column ranges in the packed [128,672]
# table column layout within the per-image [60, 896] table tensor:
#   lhs_s = u16 [60, H], rhs_s = block-diag v16 [60, 3*W]
TCOLS = dict(l1=(0, 128), r1=(128, 512), l2=(512, 576), r2=(576, 768),
             l3=(768, 800), r3=(800, 896))


class _Fallback(Exception):
    pass


def _check_grid(anchors_list):
    """Verify separable grid anchors; return per-scale (x1,x2 [W,A], y1,y2 [H,A], aa [A])."""
    out = []
    for s, anc in enumerate(anchors_list):
        H, W = SCALES[s]['H'], SCALES[s]['W']
        if anc.shape != (H * W * A, 4):
            raise _Fallback
        a4 = anc.reshape(H, W, A, 4)
        x1 = a4[0, :, :, 0]; x2 = a4[0, :, :, 2]
        y1 = a4[:, 0, :, 1]; y2 = a4[:, 0, :, 3]
        rec = np.stack([np.broadcast_to(x1[None, :, :], (H, W, A)),
                        np.broadcast_to(y1[:, None, :], (H, W, A)),
                        np.broadcast_to(x2[None, :, :], (H, W, A)),
                        np.broadcast_to(y2[:, None, :], (H, W, A))], -1)
        if not np.array_equal(rec, a4):
            raise _Fallback
        aa = (x2[0] - x1[0]) * (y2[0] - y1[0])
        if (aa <= 0).any():
            raise _Fallback
        # v = w_overlap/sqrt(c) <= aspect bound; keep p=16 powers in f32 range
        if (np.abs(x2 - x1).max() > 1e4) or (np.abs(y2 - y1).max() > 1e4):
            raise _Fallback
        out.append((x1, x2, y1, y2, aa))
    return out


def _build_tables(grids, gt_boxes):
    """[B, 60, 896] f32 matmul tables (k = a*20 + m)."""
    M = gt_boxes.shape[1]
    if M != 20:
        raise _Fallback
    gx1, gy1, gx2, gy2 = [gt_boxes[..., i] for i in range(4)]   # [B, M]
    ag = (gx2 - gx1) * (gy2 - gy1)
    import ml_dtypes
    tabs = np.zeros((B, 60, 896), ml_dtypes.bfloat16)
    for s, (x1, x2, y1, y2, aa) in enumerate(grids):
        H, W = SCALES[s]['H'], SCALES[s]['W']
        c = aa[None, :, None] + ag[:, None, :] + EPS            # [B, A, M]
        if (c <= 0).any():
            raise _Fallback
        rc = (1.0 / np.sqrt(c))[..., None]
        wint = np.clip(np.minimum(x2.T[None, :, None, :], gx2[:, None, :, None])
                       - np.maximum(x1.T[None, :, None, :], gx1[:, None, :, None]), 0, None)
        hint = np.clip(np.minimum(y2.T[None, :, None, :], gy2[:, None, :, None])
                       - np.maximum(y1.T[None, :, None, :], gy1[:, None, :, None]), 0, None)
        u = ((hint * rc) ** P16).astype(np.float32)             # [B, A, M, H]
        v = ((wint * rc) ** P16).astype(np.float32)             # [B, A, M, W]
        lc = TCOLS[f'l{s+1}']; rcols = TCOLS[f'r{s+1}']
        tabs[:, :, lc[0]:lc[1]] = u.reshape(B, 60, H)
        for a in range(A):
            c0 = rcols[0] + a * W
            tabs[:, a * 20:(a + 1) * 20, c0:c0 + W] = v[:, a]
    return tabs


def _pack_obj(preds):
    """[B, 128, 672] bf16 objectness logits; scale s at [0:H, FCOL[s]] (f=a*W+w)."""
    import ml_dtypes
    out = np.zeros((B, 128, 672), ml_dtypes.bfloat16)
    for s, sc in enumerate(SCALES):
        H, W = sc['H'], sc['W']
        pl = preds[s].reshape(B, A, 8, H, W)[:, :, 4]           # [B, A, H, W]
        out[:, :H, FCOL[s][0]:FCOL[s][1]] = pl.transpose(0, 2, 1, 3).reshape(B, H, A * W)
    return out


def _unpack_idx():
    """Per scale: [H, 3*W] global anchor ids for the natural layout."""
    out = []
    for s, sc in enumerate(SCALES):
        H, W = sc['H'], sc['W']
        h = np.arange(H)[:, None, None]
        a = np.arange(A)[None, :, None]
        w = np.arange(W)[None, None, :]
        n = NSUM[s] + (h * W + w) * A + a
        out.append(n.reshape(H, A * W))
    return out


_NC_CACHE = {}
LAST_RESULTS = None


def _build_nc():
    import concourse.bass as bass
    import concourse.tile as tile
    import concourse.mybir as mybir
    from concourse import bacc

    f32 = mybir.dt.float32
    bf16 = mybir.dt.bfloat16
    Alu = mybir.AluOpType

    nc = bacc.Bacc(None, target_bir_lowering=False)
    SH = [(128, 384), (64, 192), (32, 96)]
    PBANK = [512, 256, 128]        # per-image PSUM column stride (f32, bank-aligned)
    obj_d = nc.dram_tensor("obj", [128, BPC, 672], bf16, kind="ExternalInput")
    tab_d = nc.dram_tensor("tabs", [60, BPC, 896], bf16, kind="ExternalInput")
    excl_d = nc.dram_tensor("excl", [128, BPC, 672], bf16, kind="ExternalOutput")
    tops_d = nc.dram_tensor("tops", [128, BPC, 32], bf16, kind="ExternalOutput")

    with tile.TileContext(nc) as tc:
        with tc.tile_pool(name="sb", bufs=1) as pool, \
             tc.tile_pool(name="ps", bufs=1, space=bass.MemorySpace.PSUM) as psum:
            obj_t = pool.tile([128, BPC, 672], bf16)
            nc.sync.dma_start(obj_t[:], obj_d[:])
            tab_t = pool.tile([60, BPC, 896], bf16)
            nc.sync.dma_start(tab_t[:], tab_d[:])
            excl_t = pool.tile([128, BPC, 672], bf16)
            tops_t = pool.tile([128, BPC, 32], bf16)

            Ts = [psum.tile([SH[s][0], BPC, PBANK[s]], f32, name=f"T{s}")
                  for s in range(3)]
            for s, (H, F) in enumerate(SH):
                lc = TCOLS[f'l{s+1}']; rc = TCOLS[f'r{s+1}']
                for i in range(BPC):
                    nc.tensor.matmul(Ts[s][:, i, 0:F],
                                     tab_t[:, i, lc[0]:lc[1]],
                                     tab_t[:, i, rc[0]:rc[1]],
                                     start=True, stop=True)
            for s, (H, F) in enumerate(SH):
                f0, f1 = FCOL[s]
                # excl = -1e4 where T >= tau (exported as the candidate mask)
                nc.vector.tensor_scalar(excl_t[:H, :, f0:f1], Ts[s][:, :, 0:F],
                                        TAU_CAND, -1e4, op0=Alu.is_ge, op1=Alu.mult)
                # mining pool in place: candidate logits -> x - 1e4
                nc.vector.tensor_add(obj_t[:H, :, f0:f1], obj_t[:H, :, f0:f1],
                                     excl_t[:H, :, f0:f1])
                t0 = TOPS_OFF[s]
                for i in range(BPC):
                    for r in range(TOPS_N[s] // 8):
                        if r > 0:
                            nc.vector.match_replace(
                                out=obj_t[:H, i, f0:f1],
                                in_to_replace=tops_t[:H, i, t0 + 8*r-8:t0 + 8*r],
                                in_values=obj_t[:H, i, f0:f1], imm_value=-1e4)
                        nc.vector.max(tops_t[:H, i, t0 + 8*r:t0 + 8*r+8],
                                      obj_t[:H, i, f0:f1])
            # exact valid regions only (rows >= H per scale are never written)
            for s, (H, F) in enumerate(SH):
                f0, f1 = FCOL[s]
                t0 = TOPS_OFF[s]
                nc.sync.dma_start(excl_d[:H, :, f0:f1], excl_t[:H, :, f0:f1])
                nc.sync.dma_start(tops_d[:H, :, t0:t0 + TOPS_N[s]],
                                  tops_t[:H, :, t0:t0 + TOPS_N[s]])
    nc.finalize()
    return nc


def _run_device(objpack, tabs, trace=False):
    from concourse.bass_utils import run_bass_kernel_spmd
    global LAST_RESULTS
    if 'nc' not in _NC_CACHE:
        _NC_CACHE['nc'] = _build_nc()
    in_maps = []
    for i in range(N_CORES):
        sl = slice(i * BPC, (i + 1) * BPC)
        in_maps.append({"obj": np.ascontiguousarray(objpack[sl].transpose(1, 0, 2)),
                        "tabs": np.ascontiguousarray(tabs[sl].transpose(1, 0, 2))})
    res = run_bass_kernel_spmd(_NC_CACHE['nc'], in_maps, list(range(N_CORES)),
                               trace=trace)
    LAST_RESULTS = res
    # back to [B, 128, 672] / [B, 128, 32] image-major views
    excl = np.concatenate([np.asarray(r["excl"]).astype(np.float32).transpose(1, 0, 2)
                           for r in res.results], 0)
    tops = np.concatenate([np.asarray(r["tops"]).astype(np.float32).transpose(1, 0, 2)
                           for r in res.results], 0)
    masks = [excl[:, :SCALES[s]['H'], FCOL[s][0]:FCOL[s][1]] < -1.0 for s in range(3)]
    tops_l = [tops[:, :SCALES[s]['H'], TOPS_OFF[s]:TOPS_OFF[s] + TOPS_N[s]]
              for s in range(3)]
    return masks, tops_l


def _softplus(x):
    return np.log1p(np.exp(-np.abs(x))) + np.maximum(x, 0)


def _host_finish(inputs, masks, tops_all):
    anchors = np.concatenate([inputs[f'anchors{i}'] for i in (1, 2, 3)], 0)
    aa = (anchors[:, 2] - anchors[:, 0]) * (anchors[:, 3] - anchors[:, 1])
    idx_maps = _unpack_idx()
    preds = [inputs['pred1'], inputs['pred2'], inputs['pred3']]
    pflat = [p.reshape(B, 24, -1) for p in preds]
    obj_sum = 0.0; obj_den = 0; cls_sum = 0.0; loc_sum = 0.0; n_pos_t = 0
    for b in range(B):
        gt = inputs['gt_boxes'][b]; lab = inputs['gt_labels'][b]
        ag = (gt[:, 2] - gt[:, 0]) * (gt[:, 3] - gt[:, 1])
        cand_n = np.concatenate([idx_maps[s][masks[s][b] > 0] for s in range(3)])
        ca = anchors[cand_n]
        lt = np.maximum(ca[:, None, :2], gt[None, :, :2])
        rb = np.minimum(ca[:, None, 2:], gt[None, :, 2:])
        wh = np.clip(rb - lt, 0, None)
        inter = wh[..., 0] * wh[..., 1]
        iou = inter / (aa[cand_n][:, None] + ag[None, :] - inter + EPS)
        bi = iou.max(1) if cand_n.size else np.empty(0)
        bg = iou.argmax(1) if cand_n.size else np.empty(0, np.int64)
        pos_c = bi >= 0.5
        nonneg_c = bi >= 0.3
        pos_n = cand_n[pos_c]; pos_bg = bg[pos_c]
        n_pos = pos_n.size

        def gather(ns, chans):
            out = np.empty((len(chans), ns.size), np.float32)
            for s in range(3):
                m = (ns >= NSUM[s]) & (ns < NSUM[s + 1])
                if not m.any():
                    continue
                loc = ns[m] - NSUM[s]
                a = loc % A; hw = loc // A
                for ci, c in enumerate(chans):
                    out[ci, m] = pflat[s][b][a * 8 + c, hw]
            return out

        for s in range(3):
            in_s = (cand_n >= NSUM[s]) & (cand_n < NSUM[s + 1])
            n_pos_s = int((pos_c & in_s).sum())
            k = 3 * max(n_pos_s, 1)
            n_neg_s = (NSUM[s + 1] - NSUM[s]) - int((nonneg_c & in_s).sum())
            k_eff = min(k, n_neg_s)
            cn = cand_n[(~nonneg_c) & in_s]
            extra = _softplus(gather(cn, [4])[0]) if cn.size else np.empty(0, np.float32)
            tseg = tops_all[s][b]                # raw logits, -1e4 = excluded
            merged = np.concatenate([_softplus(tseg[tseg > -1e3].ravel()), extra])
            sel = np.sort(merged)[::-1][:k_eff]
            tstar = sel[-1] if (k_eff > 0 and sel.size == k_eff) else np.inf
            if (sel.size < k_eff) or (not np.isfinite(sel.sum())) \
                    or np.any(_softplus(tseg[:, -1]) >= tstar):
                # exact fallback for this (image, scale)
                x_all = pflat[s][b][[4, 12, 20]].T.ravel()      # n_local = hw*A + a
                sp_all = _softplus(x_all)
                negm = np.ones(NSUM[s + 1] - NSUM[s], bool)
                negm[cand_n[nonneg_c & in_s] - NSUM[s]] = False
                sel = np.sort(sp_all[negm])[::-1][:k_eff]
            obj_sum += float(sel.sum())
            obj_den += n_pos_s + k_eff

        if n_pos:
            pv = gather(pos_n, [4, 5, 6, 7, 0, 1, 2, 3])
            x = pv[0]
            obj_sum += float((_softplus(x) - x).sum())
            logits = pv[1:4]
            mlog = logits.max(0)
            lse = mlog + np.log(np.exp(logits - mlog).sum(0))
            tgt = np.clip(lab[pos_bg], 0, C - 1)
            cls_sum += float((lse - logits[tgt, np.arange(n_pos)]).sum())
            mb = gt[pos_bg]; anc = anchors[pos_n]

            def cxcywh(bx):
                w = np.maximum(bx[:, 2] - bx[:, 0], EPS)
                h = np.maximum(bx[:, 3] - bx[:, 1], EPS)
                return bx[:, 0] + 0.5 * w, bx[:, 1] + 0.5 * h, w, h

            gcx, gcy, gw, gh = cxcywh(mb)
            acx, acy, aw, ah = cxcywh(anc)
            t = np.stack([(gcx - acx) / (aw + EPS), (gcy - acy) / (ah + EPS),
                          np.log((gw + EPS) / (aw + EPS)),
                          np.log((gh + EPS) / (ah + EPS))])
            d = pv[4:8] - t
            ad = np.abs(d)
            loc_sum += float(np.where(ad < 1, 0.5 * d * d, ad - 0.5).sum())
        n_pos_t += n_pos

    pos_norm = max(n_pos_t, 1); obj_norm = max(obj_den, 1)
    lo = obj_sum / obj_norm; lc = cls_sum / pos_norm; ll = loc_sum / pos_norm
    return np.array([lo, lc, ll, lo + lc + 2 * ll], np.float32)


def _kernel_numpy(pred1, pred2, pred3, anchors1, anchors2, anchors3,
                  gt_boxes, gt_labels):
    """Exact reference-equivalent numpy fallback (arbitrary inputs)."""
    tot = [0.0, 0, 0.0, 0.0, 0]
    for pred, anc in ((pred1, anchors1), (pred2, anchors2), (pred3, anchors3)):
        Bb, ch, H, W = pred.shape
        p = pred.reshape(Bb, A, 5 + C, H, W).transpose(0, 3, 4, 1, 2).reshape(Bb, -1, 5 + C)
        N = p.shape[1]
        aa = (anc[:, 2] - anc[:, 0]) * (anc[:, 3] - anc[:, 1])
        for b in range(Bb):
            boxes = gt_boxes[b]; labels = gt_labels[b]
            ag = (boxes[:, 2] - boxes[:, 0]) * (boxes[:, 3] - boxes[:, 1])
            lt = np.maximum(anc[:, None, :2], boxes[None, :, :2])
            rb = np.minimum(anc[:, None, 2:], boxes[None, :, 2:])
            wh = np.clip(rb - lt, 0, None)
            inter = wh[..., 0] * wh[..., 1]
            ious = inter / (aa[:, None] + ag[None, :] - inter + EPS)
            bi = ious.max(1); bg = ious.argmax(1)
            pos = bi >= 0.5; neg = bi < 0.3
            x = p[b, :, 4]
            ol = np.maximum(x, 0) - x * pos + np.log1p(np.exp(-np.abs(x)))
            k = 3 * max(int(pos.sum()), 1)
            nl = np.where(neg, ol, -np.inf)
            order = np.argsort(-nl, kind='stable')
            rank = np.empty(N, np.int64); rank[order] = np.arange(N)
            seln = (rank < k) & neg
            m = pos | seln
            tot[0] += float(ol[m].sum()); tot[1] += int(m.sum())
            if pos.any():
                logits = p[b, pos, 5:]
                mlog = logits.max(1, keepdims=True)
                lse = (mlog[:, 0] + np.log(np.exp(logits - mlog).sum(1)))
                tgt = np.clip(labels[bg[pos]], 0, C - 1)
                tot[2] += float((lse - logits[np.arange(tgt.size), tgt]).sum())
                mb = boxes[bg[pos]]; ap_ = anc[pos]

                def cxcywh(bx):
                    w = np.maximum(bx[:, 2] - bx[:, 0], EPS)
                    h = np.maximum(bx[:, 3] - bx[:, 1], EPS)
                    return bx[:, 0] + 0.5 * w, bx[:, 1] + 0.5 * h, w, h

                gcx, gcy, gw, gh = cxcywh(mb); acx, acy, aw, ah = cxcywh(ap_)
                t = np.stack([(gcx - acx) / (aw + EPS), (gcy - acy) / (ah + EPS),
                              np.log((gw + EPS) / (aw + EPS)),
                              np.log((gh + EPS) / (ah + EPS))], 1)
                d = p[b, pos, :4] - t
                ad = np.abs(d)
                tot[3] += float(np.where(ad < 1, 0.5 * d * d, ad - 0.5).sum())
                tot[4] += int(pos.sum())
    lo = tot[0] / max(tot[1], 1); lc = tot[2] / max(tot[4], 1)
    ll = tot[3] / max(tot[4], 1)
    return np.array([lo, lc, ll, lo + lc + 2 * ll], np.float32)


def kernel(pred1, pred2, pred3, anchors1, anchors2, anchors3,
           gt_boxes, gt_labels, _trace=False):
    args = dict(pred1=np.asarray(pred1, np.float32),
                pred2=np.asarray(pred2, np.float32),
                pred3=np.asarray(pred3, np.float32),
                anchors1=np.asarray(anchors1, np.float32),
                anchors2=np.asarray(anchors2, np.float32),
                anchors3=np.asarray(anchors3, np.float32),
                gt_boxes=np.asarray(gt_boxes, np.float32),
                gt_labels=np.asarray(gt_labels))
    try:
        if args['pred1'].shape != (B, 24, 128, 128) or args['gt_boxes'].shape != (B, 20, 4):
            raise _Fallback
        grids = _check_grid([args[f'anchors{i}'] for i in (1, 2, 3)])
        tabs = _build_tables(grids, args['gt_boxes'])
    except _Fallback:
        return _kernel_numpy(**{k: v for k, v in args.items()})
    objpack = _pack_obj([args[f'pred{i}'] for i in (1, 2, 3)])
    masks, tops = _run_device(objpack, tabs, trace=_trace)
    return _host_finish(args, masks, tops)
